# revision 1
# baseline (speedup 1.0000x reference)
"""AdvancedFeatureTransformer Trainium2 kernel.

Strategy: pure data-parallel over batch (8 cores x 512 rows), no collectives.
All activations feature-major (h^T: [feat_part, batch_free]) so every matmul
streams the batch as the N=512 moving operand at full PE rate.

Host preprocessing (free, small weights):
  - every weight feeding a LayerNorm input is centered over its output axis
    (W - mean, b - mean).  Combined with ln_g==1/ln_b==0 (checked), every LN
    input has exactly zero feature-mean, so only sum-of-squares is needed on
    device (one ones-matmul per LN row block).
  - all weights pre-transposed to [in, out] so they DMA directly as lhsT.
  - head W3 packed as 2-target block-diagonal lhsT; W2/b2 pair-packed.

Per-target head pipeline (feature-major [128f, 512b]):
  W1 matmul(x2) -> psum; ACT: T = psum + b1c; DVE: sq = T*T;
  PE: sumsq row = ones^T @ sq; GPSIMD: R = relu(T)   [relu commutes with the
  positive rstd scale, which is applied after W2 instead];
  PE: Z = W2^T-pair @ R; DVE: U = Z * bcast(rstd); ACT: R2 = relu(U + b2);
  PE: out row-pair = blockdiag(W3) @ R2.
"""

import sys

if "/opt/trn_rl_repo" not in sys.path:
    sys.path.insert(0, "/opt/trn_rl_repo")

import numpy as np

B = 4096
NCORES = 8
BL = B // NCORES        # 512 rows per core
DIN = 512
D = 256
T = 424
L = 6
EPS = 1e-5
PAIRS = T // 2          # 212
SUB = 16                # stats sub-block (targets per rstd batch)

_cache = {}


def _prep(inputs):
    """Host-side weight preprocessing -> per-core input map (shared arrays)."""
    f32 = lambda a: np.ascontiguousarray(np.asarray(a, dtype=np.float32))

    x = f32(inputs["x"])
    # trunk LN scale/bias must be identity for the zero-mean centering trick
    assert np.all(np.asarray(inputs["ln_g"]) == 1.0), "ln_g != 1 unsupported"
    assert np.all(np.asarray(inputs["ln_b"]) == 0.0), "ln_b != 0 unsupported"
    assert np.all(np.asarray(inputs["tp_ln_g"]) == 1.0), "tp_ln_g != 1 unsupported"
    assert np.all(np.asarray(inputs["tp_ln_b"]) == 0.0), "tp_ln_b != 0 unsupported"

    # ---- projection (centered over output axis) ----
    Wp = f32(inputs["proj_W"]).reshape(D, DIN)        # [256, 512]
    bp = f32(inputs["proj_b"]).reshape(D)
    WpT = f32(Wp.T)                                   # [512, 256] (raw!)
    wpm = f32(Wp.mean(0))                             # [512]  row-mean weights
    wpmC = f32(wpm.reshape(4, 128).T)                 # [128, 4] k-chunk cols
    bpm = float(bp.mean())

    # ---- trunk layers ----
    aiW = f32(inputs["attn_in_W"])                    # [6, 768, 256]
    aib = f32(inputs["attn_in_b"])                    # [6, 768]
    aoW = f32(inputs["attn_out_W"])                   # [6, 256, 256]
    aob = f32(inputs["attn_out_b"])
    f1W = f32(inputs["ff_W1"])                        # [6, 1024, 256]
    f1b = f32(inputs["ff_b1"])
    f2W = f32(inputs["ff_W2"])                        # [6, 256, 1024]
    f2b = f32(inputs["ff_b2"])

    WvT = np.empty((L, D, D), np.float32)
    WoT = np.empty((L, D, D), np.float32)
    Wf1T = np.empty((L, D, 4 * D), np.float32)
    Wf2T = np.empty((L, 4 * D, D), np.float32)
    for i in range(L):
        Wv = aiW[i, 2 * D:]                            # [256, 256]
        WvT[i] = Wv.T
        Wo = aoW[i] - aoW[i].mean(0, keepdims=True)    # center attn output
        WoT[i] = Wo.T
        Wf1T[i] = f1W[i].T
        Wf2 = f2W[i] - f2W[i].mean(0, keepdims=True)   # center ff output
        Wf2T[i] = Wf2.T
    bv = f32(aib[:, 2 * D:])                           # [6, 256]
    bo = f32(aob - aob.mean(1, keepdims=True))
    bf1 = f32(f1b)                                     # [6, 1024]
    bf2 = f32(f2b - f2b.mean(1, keepdims=True))

    # ---- cross attention (no LN after -> no centering) ----
    cW = f32(inputs["cross_in_W"])
    WcvT = f32(cW[2 * D:].T)                           # [256, 256]
    bcv = f32(inputs["cross_in_b"])[2 * D:]
    WcoT = f32(f32(inputs["cross_out_W"]).T)
    bco = f32(inputs["cross_out_b"])

    # ---- trunk bias pack: one [nb,128] array -> sbuf [128, nb] ----
    cols = []

    def pack(vec):  # returns starting column index
        v = f32(vec).reshape(-1, 128)
        s = len(cols)
        cols.extend(v)
        return s

    bias_idx = {
        "bp": pack(bp),
        "bv": [pack(bv[i]) for i in range(L)],
        "bo": [pack(bo[i]) for i in range(L)],
        "bf1": [pack(bf1[i]) for i in range(L)],
        "bf2": [pack(bf2[i]) for i in range(L)],
        "bcv": pack(bcv),
        "bco": pack(bco),
    }
    TB = f32(np.stack(cols))                           # [nb, 128]

    # ---- heads ----
    W1 = f32(inputs["tp_W1"])                          # [424, 128, 256]
    b1 = f32(inputs["tp_b1"])                          # [424, 128]
    W1c = W1 - W1.mean(1, keepdims=True)
    b1c = b1 - b1.mean(1, keepdims=True)
    W1T = f32(W1c.transpose(0, 2, 1))                  # [424, 256, 128]
    b1T = f32(b1c.T)                                   # [128, 424]

    W2 = f32(inputs["tp_W2"])                          # [424, 64, 128]
    b2 = f32(inputs["tp_b2"])                          # [424, 64]
    W2P = f32(W2.transpose(0, 2, 1).reshape(PAIRS, 2, 128, 64))
    b2P = f32(b2.reshape(PAIRS, 128).T)                # [128, 212]

    W3 = f32(inputs["tp_W3"])                          # [424, 64]
    b3 = f32(inputs["tp_b3"])                          # [424]
    NG = (T + 31) // 32                                # 14 groups of <=32 targets
    W3BD = np.zeros((PAIRS, 128, 32), np.float32)
    for p in range(PAIRS):
        q = p % 16                                     # pair index within group
        W3BD[p, 0:64, 2 * q] = W3[2 * p]
        W3BD[p, 64:128, 2 * q + 1] = W3[2 * p + 1]
    b3B = np.zeros((32, NG), np.float32)
    for t in range(T):
        b3B[t % 32, t // 32] = b3[t]

    shared = {
        "WpT": WpT, "wpmC": wpmC, "WvT": WvT, "WoT": WoT, "Wf1T": Wf1T, "Wf2T": Wf2T,
        "WcvT": WcvT, "WcoT": WcoT, "TB": TB,
        "W1T": W1T, "b1T": b1T, "W2P": W2P, "b2P": b2P,
        "W3BD": W3BD, "b3B": b3B,
    }
    in_maps = []
    for c in range(NCORES):
        m = dict(shared)
        m["xT"] = f32(x[c * BL:(c + 1) * BL].T)        # [512 din, 512 b]
        in_maps.append(m)
    return in_maps, TB.shape[0], bias_idx, bpm


def _build(nb, bias_idx, bpm):
    import concourse.bass as bass
    import concourse.mybir as mybir
    import concourse.tile as tile
    from concourse import bacc
    from concourse.masks import make_identity

    dt = mybir.dt.float32
    Alu = mybir.AluOpType
    Act = mybir.ActivationFunctionType
    ts = bass.ts

    nc = bacc.Bacc(None, target_bir_lowering=False)
    dr = lambda name, shape: nc.dram_tensor(name, shape, dt, kind="ExternalInput")
    xT = dr("xT", [DIN, BL])
    WpT = dr("WpT", [DIN, D])
    wpmC = dr("wpmC", [128, 4])
    WvT = dr("WvT", [L, D, D])
    WoT = dr("WoT", [L, D, D])
    Wf1T = dr("Wf1T", [L, D, 4 * D])
    Wf2T = dr("Wf2T", [L, 4 * D, D])
    WcvT = dr("WcvT", [D, D])
    WcoT = dr("WcoT", [D, D])
    TB = dr("TB", [nb, 128])
    W1T = dr("W1T", [T, D, 128])
    b1T = dr("b1T", [128, T])
    W2P = dr("W2P", [PAIRS, 2, 128, 64])
    b2P = dr("b2P", [128, PAIRS])
    W3BD = dr("W3BD", [PAIRS, 128, 32])
    b3B = dr("b3B", [32, 14])
    out = nc.dram_tensor("out", [BL, T], dt, kind="ExternalOutput")

    from contextlib import ExitStack

    with tile.TileContext(nc) as tc, ExitStack() as stack:
        consts = stack.enter_context(tc.tile_pool(name="consts", bufs=1))
        hpool = stack.enter_context(tc.tile_pool(name="hpool", bufs=4))

        # constants
        ones_col = consts.tile([128, 1], dt, tag="ones")
        nc.vector.memset(ones_col, 1.0)
        eps_col = consts.tile([128, 1], dt, tag="eps")
        nc.vector.memset(eps_col, EPS)
        ones64 = consts.tile([128, 64], dt, tag="ones64")
        nc.vector.memset(ones64, 1.0)
        idn = consts.tile([128, 128], dt, tag="idn")
        make_identity(nc, idn)
        tb_sb = consts.tile([128, nb], dt, tag="tb")
        nc.gpsimd.dma_start(out=tb_sb, in_=TB.rearrange("n p -> p n"))
        b1_sb = consts.tile([128, T], dt, tag="b1")
        nc.gpsimd.dma_start(out=b1_sb, in_=b1T[:, :])
        b2_sb = consts.tile([128, PAIRS], dt, tag="b2")
        nc.gpsimd.dma_start(out=b2_sb, in_=b2P[:, :])
        b3_sb = consts.tile([32, 14], dt, tag="b3")
        nc.gpsimd.dma_start(out=b3_sb, in_=b3B[:, :])
        out_sb = [consts.tile([128, T], dt, tag=f"ob{i}", name=f"ob{i}")
                  for i in range(4)]

        def stt(o, a, s, b, op0, op1, **kw):
            nc.vector.scalar_tensor_tensor(
                out=o, in0=a, scalar=s, in1=b, op0=op0, op1=op1, **kw)

        def bias_col(idx, m=0):
            return tb_sb[:, idx + m:idx + m + 1]

        # ---------------- trunk ----------------
        with tc.tile_pool(name="twt", bufs=2) as twt, \
             tc.tile_pool(name="tact", bufs=2) as tact, \
             tc.tile_pool(name="tps", bufs=4, space="PSUM") as tps, \
             tc.tile_pool(name="tpss", bufs=2, space="PSUM") as tpss:

            xs = twt.tile([128, 4, BL], dt, tag="x")
            nc.gpsimd.dma_start(out=xs, in_=xT.rearrange("(c k) b -> k c b", c=4))
            wp = twt.tile([128, 4, D], dt, tag="wp")
            nc.gpsimd.dma_start(out=wp, in_=WpT.rearrange("(c k) m -> k c m", c=4))
            wpm_sb = twt.tile([128, 4], dt, tag="wpm")
            nc.gpsimd.dma_start(out=wpm_sb, in_=wpmC[:, :])
            cps = tpss.tile([1, BL], dt, tag="cps")
            for k in range(4):
                nc.tensor.matmul(cps, wpm_sb[:, k:k + 1], xs[:, k],
                                 start=(k == 0), stop=(k == 3))
            c_sb = tact.tile([1, BL], dt, tag="c1")
            nc.scalar.activation(c_sb, cps, Act.Identity, bias=eps_col[0:1],
                                 scale=1.0)
            c_bc = consts.tile([128, BL], dt, tag="cbc")
            nc.gpsimd.partition_broadcast(c_bc, c_sb, channels=128)

            h = [hpool.tile([128, BL], dt, tag=f"h{m}", name=f"h{m}")
                 for m in range(2)]
            for m in range(2):
                ps = tps.tile([128, BL], dt, tag="mm")
                for k in range(4):
                    nc.tensor.matmul(ps, wp[:, k, ts(m, 128)], xs[:, k],
                                     start=(k == 0), stop=(k == 3))
                nc.vector.tensor_scalar_add(h[m], ps, bias_col(bias_idx["bp"], m))

            def layer_norm(y, sub_mean=None, sub_const=0.0):
                """y: 2 feature-major [128,BL] tiles -> normalized.
                If sub_mean given: y is first centered by (y - sub_const) - sub_mean."""
                if sub_mean is not None:
                    yc = []
                    for k in range(2):
                        t2 = tact.tile([128, BL], dt, tag=f"yc{k}", name="yc")
                        stt(t2, y[k], sub_const, sub_mean,
                            Alu.subtract, Alu.subtract)
                        yc.append(t2)
                    y = yc
                ssq = tpss.tile([1, BL], dt, tag="ssq")
                for k in range(2):
                    sq = tact.tile([128, BL], dt, tag="sq")
                    stt(sq, y[k], 1.0, y[k], Alu.mult, Alu.mult)
                    nc.tensor.matmul(ssq, ones_col, sq,
                                     start=(k == 0), stop=(k == 1))
                sd = tact.tile([1, BL], dt, tag="sd")
                nc.scalar.activation(sd, ssq, Act.Sqrt,
                                     bias=eps_col[0:1], scale=1.0 / D)
                r = tact.tile([1, BL], dt, tag="rr")
                nc.vector.reciprocal_approx_fast(out=r, in_=sd)
                rb = tact.tile([128, BL], dt, tag="rb")
                nc.gpsimd.partition_broadcast(rb, r, channels=128)
                hn = [hpool.tile([128, BL], dt, tag=f"h{m}", name=f"hn{m}")
                      for m in range(2)]
                for m in range(2):
                    stt(hn[m], y[m], 1.0, rb, Alu.mult, Alu.mult)
                return hn

            def proj2(wt_dram, h_in, tag):
                """[256->256] matmul, returns psum tiles (2)."""
                w = twt.tile([128, 2, D], dt, tag=tag)
                nc.gpsimd.dma_start(out=w, in_=wt_dram.rearrange(
                    "(c k) m -> k c m", c=2))
                pss = []
                for m in range(2):
                    ps = tps.tile([128, BL], dt, tag="mm")
                    for k in range(2):
                        nc.tensor.matmul(ps, w[:, k, ts(m, 128)], h_in[k],
                                         start=(k == 0), stop=(k == 1))
                    pss.append(ps)
                return pss

            for i in range(L):
                # attention == out_proj(v_proj(h))
                vps = proj2(WvT[i], h, "wv")
                v = []
                for m in range(2):
                    vt = tact.tile([128, BL], dt, tag=f"v{m}")
                    nc.scalar.activation(vt, vps[m], Act.Identity,
                                         bias=bias_col(bias_idx["bv"][i], m),
                                         scale=1.0)
                    v.append(vt)
                aps = proj2(WoT[i], v, "wo")
                y = []
                for m in range(2):
                    yt = tact.tile([128, BL], dt, tag=f"y{m}")
                    stt(yt, aps[m], bias_col(bias_idx["bo"][i], m), h[m],
                        Alu.add, Alu.add)
                    y.append(yt)
                if i == 0:
                    h = layer_norm(y, sub_mean=c_bc, sub_const=bpm)
                else:
                    h = layer_norm(y)

                # feed-forward
                w1 = twt.tile([128, 2, 4 * D], dt, tag="wf1")
                nc.gpsimd.dma_start(out=w1, in_=Wf1T[i].rearrange(
                    "(c k) m -> k c m", c=2))
                g = []
                for m in range(8):
                    ps = tps.tile([128, BL], dt, tag="mm")
                    for k in range(2):
                        nc.tensor.matmul(ps, w1[:, k, ts(m, 128)], h[k],
                                         start=(k == 0), stop=(k == 1))
                    gt = tact.tile([128, BL], dt, tag=f"g{m}")
                    nc.scalar.activation(gt, ps, Act.Gelu,
                                         bias=bias_col(bias_idx["bf1"][i], m),
                                         scale=1.0)
                    g.append(gt)
                w2 = twt.tile([128, 8, D], dt, tag="wf2")
                nc.gpsimd.dma_start(out=w2, in_=Wf2T[i].rearrange(
                    "(c k) m -> k c m", c=8))
                y = []
                for m in range(2):
                    ps = tps.tile([128, BL], dt, tag="mm")
                    for k in range(8):
                        nc.tensor.matmul(ps, w2[:, k, ts(m, 128)], g[k],
                                         start=(k == 0), stop=(k == 7))
                    yt = tact.tile([128, BL], dt, tag=f"y{m}")
                    stt(yt, ps, bias_col(bias_idx["bf2"][i], m), h[m],
                        Alu.add, Alu.add)
                    y.append(yt)
                h = layer_norm(y)

            # cross attention (v/out only, residual, no LN)
            vps = proj2(WcvT, h, "wv")
            v = []
            for m in range(2):
                vt = tact.tile([128, BL], dt, tag=f"v{m}")
                nc.scalar.activation(vt, vps[m], Act.Identity,
                                     bias=bias_col(bias_idx["bcv"], m), scale=1.0)
                v.append(vt)
            aps = proj2(WcoT, v, "wo")
            hc = [consts.tile([128, BL], dt, tag=f"hc{m}", name=f"hc{m}")
                  for m in range(2)]
            for m in range(2):
                stt(hc[m], aps[m], bias_col(bias_idx["bco"], m), h[m],
                    Alu.add, Alu.add)

        # ---------------- heads ----------------
        with tc.tile_pool(name="hwt", bufs=4) as hwt, \
             tc.tile_pool(name="hT", bufs=3) as hT, \
             tc.tile_pool(name="hR", bufs=6) as hR, \
             tc.tile_pool(name="hrb", bufs=3) as hrb, \
             tc.tile_pool(name="hps", bufs=2, space="PSUM") as hps, \
             tc.tile_pool(name="hpw", bufs=2, space="PSUM") as hpw, \
             tc.tile_pool(name="hpst", bufs=2, space="PSUM") as hpst, \
             tc.tile_pool(name="hpo", bufs=1, space="PSUM") as hpo, \
             tc.tile_pool(name="hpt", bufs=1, space="PSUM") as hpt:

            zero_sb = consts.tile([128, BL], dt, tag="zero")
            nc.vector.memset(zero_sb, 0.0)
            for blk in range(4):
                bs = min(128, T - blk * 128)
                t0b = blk * 128
                o3 = hT.tile([128, BL], dt, tag="o3s", name="o3")
                for g in range((bs + 31) // 32):       # 32-target groups
                    gs = min(32, bs - g * 32)
                    gg = blk * 4 + g                   # global group index
                    o3g = hpo.tile([32, BL], dt, tag="o3g", name="o3g")
                    npair = gs // 2
                    for q in range(npair):
                        pg = gg * 16 + q               # global pair index
                        tA = 2 * pg
                        # --- per-target: W1 matmul, T, sq, relu ---
                        Ts, sqs, Rs = [], [], []
                        for e in range(2):
                            t = tA + e
                            w1 = hwt.tile([128, 2, 128], dt, tag="w1", name="w1")
                            nc.gpsimd.dma_start(out=w1, in_=W1T[t].rearrange(
                                "(c k) m -> k c m", c=2))
                            sps = hps.tile([128, BL], dt, tag="s", name="sps")
                            for k in range(2):
                                nc.tensor.matmul(sps, w1[:, k], hc[k],
                                                 start=(k == 0), stop=(k == 1))
                            Tt = hT.tile([128, BL], dt, tag="T", name="Tt")
                            nc.scalar.activation(Tt, sps, Act.Identity,
                                                 bias=b1_sb[:, t:t + 1], scale=1.0)
                            sq = hT.tile([128, BL], dt, tag="sq", name="sq")
                            stt(sq, Tt, 1.0, Tt, Alu.mult, Alu.mult)
                            R = hR.tile([128, BL], dt, tag="R", name="R")
                            nc.gpsimd.tensor_scalar_max(R, Tt, 0.0)
                            sqs.append(sq)
                            Rs.append(R)
                        # --- pair stats: sumsq replicated into halves ---
                        stp = hpst.tile([128, BL], dt, tag="st", name="stp")
                        nc.tensor.matmul(stp[0:64], ones64, sqs[0],
                                         start=True, stop=True)
                        nc.tensor.matmul(stp[64:128], ones64, sqs[1],
                                         start=True, stop=True)
                        sd = hrb.tile([128, BL], dt, tag="sd", name="sd")
                        nc.scalar.activation(sd, stp, Act.Sqrt,
                                             bias=eps_col, scale=1.0 / 128)
                        rb = hrb.tile([128, BL], dt, tag="rb", name="rb")
                        nc.vector.reciprocal_approx_fast(out=rb, in_=sd)
                        # --- W2 pair, scale by rstd, relu(+b2), W3 ---
                        w2 = hwt.tile([128, 2, 64], dt, tag="w2", name="w2")
                        nc.gpsimd.dma_start(out=w2, in_=W2P[pg].rearrange(
                            "e k m -> k e m"))
                        zps = hpw.tile([128, BL], dt, tag="z", name="zps")
                        nc.tensor.matmul(zps[0:64], w2[:, 0], Rs[0],
                                         start=True, stop=True)
                        nc.tensor.matmul(zps[64:128], w2[:, 1], Rs[1],
                                         start=True, stop=True)
                        U = hT.tile([128, BL], dt, tag="U", name="U")
                        stt(U, zps, 1.0, rb, Alu.mult, Alu.mult)
                        R2 = hT.tile([128, BL], dt, tag="R2", name="R2")
                        nc.gpsimd.tensor_scalar(
                            out=R2, in0=U, scalar1=b2_sb[:, pg:pg + 1],
                            scalar2=0.0, op0=Alu.add, op1=Alu.max)
                        w3 = hwt.tile([128, 32], dt, tag="w3", name="w3")
                        nc.gpsimd.dma_start(out=w3, in_=W3BD[pg])
                        nc.tensor.matmul(o3g, w3, R2,
                                         start=(q == 0), stop=(q == npair - 1))
                    # group done: bias + move to block sbuf tile
                    nc.scalar.activation(o3[g * 32:g * 32 + gs], o3g[0:gs],
                                         Act.Identity,
                                         bias=b3_sb[0:gs, gg:gg + 1], scale=1.0)
                # block done: transpose [bs,512] -> [512,bs]
                for bc in range(4):
                    tp = hpt.tile([128, 128], dt, tag="tp", name="tp")
                    nc.tensor.transpose(tp[:, 0:bs], o3[0:bs, ts(bc, 128)],
                                        idn[0:bs, 0:bs])
                    nc.vector.tensor_copy(
                        out_sb[bc][:, t0b:t0b + bs], tp[:, 0:bs])
            for bc in range(4):
                nc.gpsimd.dma_start(out=out[ts(bc, 128)], in_=out_sb[bc])

    nc.compile()
    return nc


def kernel(**inputs):
    from concourse.bass_utils import run_bass_kernel_spmd

    in_maps, nb, bias_idx, bpm = _prep(inputs)
    if "nc" not in _cache:
        _cache["nc"] = _build(nb, bias_idx, bpm)
    nc = _cache["nc"]
    import os
    res = run_bass_kernel_spmd(
        nc, in_maps, core_ids=list(range(NCORES)),
        trace=bool(int(os.environ.get("KTRACE", "0"))))
    _cache["last_result"] = res
    outs = [r["out"] for r in res.results]
    return np.concatenate(outs, axis=0)



# revision 6
# speedup vs baseline: 7.6241x; 7.6241x over previous
"""AdvancedFeatureTransformer Trainium2 kernel (bf16, engine-balanced).

Data-parallel over batch: 8 cores x 512 rows, no collectives.
All activations feature-major (h^T: [feat_part, batch_free]); every matmul
operand is bf16 (1 cycle/row on the PE vs 4 for fp32), psum accumulates fp32.

Key structural choices (driven by a HW trace of the fp32 baseline):
  - NO gpsimd elementwise (measured 7.5us per [128,512] op); gpsimd only
    issues DMAs and the rare partition_broadcast.
  - Heads read每 psum exactly twice (ACT Square for LN stats, DVE
    tensor_scalar relu) -- no materialized pre-activation.
  - LN rstd is algebraically deferred:
       relu(Z*r + b2) = r * relu(Z + b2*sd),  sd = 1/r
    The rank-1 b2 (x) sd term is added into the W2 psum by a K=32
    selection matmul against a group-level sd tile [32t, 512b]; the final
    r scale is applied once per 32-target group on the W3 output.
  - LN stats are computed by selection-matrix matmuls (M=32) accumulating
    into one [32, 512] psum bank per group, so sqrt/recip run once per 32
    targets instead of per pair.
  - Trunk residuals are folded into the out-projection psum with an
    identity matmul, and biases ride the ACT bias port, eliminating all
    3-operand DVE ops (2.4us each measured) except one pair in layer 0.
  - Weight DMAs are host-packed into large contiguous blocks (one DMA per
    8 targets / 8 pairs / group) to cut software-DGE instruction count.
"""

import sys

if "/opt/trn_rl_repo" not in sys.path:
    sys.path.insert(0, "/opt/trn_rl_repo")

import numpy as np
import ml_dtypes

BF16 = ml_dtypes.bfloat16

B = 4096
NCORES = 8
BL = B // NCORES        # 512 rows per core
DIN = 512
D = 256
T = 424
L = 6
EPS = 1e-5
PAIRS = T // 2          # 212
NG = (T + 31) // 32     # 14 head groups of <=32 targets
NB8 = T // 8            # 53 W1 dma blocks of 8 targets
NPB = 27                # W2 dma blocks of 8 pairs (216 padded)

_cache = {}


def _prep(inputs):
    f32 = lambda a: np.ascontiguousarray(np.asarray(a, dtype=np.float32))
    b16 = lambda a: np.ascontiguousarray(np.asarray(a, dtype=np.float32).astype(BF16))

    x = f32(inputs["x"])
    assert np.all(np.asarray(inputs["ln_g"]) == 1.0), "ln_g != 1 unsupported"
    assert np.all(np.asarray(inputs["ln_b"]) == 0.0), "ln_b != 0 unsupported"
    assert np.all(np.asarray(inputs["tp_ln_g"]) == 1.0), "tp_ln_g != 1 unsupported"
    assert np.all(np.asarray(inputs["tp_ln_b"]) == 0.0), "tp_ln_b != 0 unsupported"

    # ---- projection ----
    Wp = f32(inputs["proj_W"]).reshape(D, DIN)        # [256, 512]
    bp = f32(inputs["proj_b"]).reshape(D)
    WpT = b16(Wp.T)                                   # [512, 256] raw
    wpm = Wp.mean(0)                                  # [512] col means
    wpmC = b16(wpm.reshape(4, 128).T)                 # [128, 4]
    bpm = float(bp.mean())

    # ---- trunk ----
    aiW = f32(inputs["attn_in_W"])
    aib = f32(inputs["attn_in_b"])
    aoW = f32(inputs["attn_out_W"])
    aob = f32(inputs["attn_out_b"])
    f1W = f32(inputs["ff_W1"])
    f1b = f32(inputs["ff_b1"])
    f2W = f32(inputs["ff_W2"])
    f2b = f32(inputs["ff_b2"])

    WvT = np.empty((L, D, D), np.float32)
    WoT = np.empty((L, D, D), np.float32)
    Wf1T = np.empty((L, D, 4 * D), np.float32)
    Wf2T = np.empty((L, 4 * D, D), np.float32)
    for i in range(L):
        WvT[i] = aiW[i, 2 * D:].T
        WoT[i] = (aoW[i] - aoW[i].mean(0, keepdims=True)).T
        Wf1T[i] = f1W[i].T
        Wf2T[i] = (f2W[i] - f2W[i].mean(0, keepdims=True)).T
    bv = aib[:, 2 * D:]
    bo = aob - aob.mean(1, keepdims=True)
    bo0p = bo[0] - bpm                                 # layer-0 merged bias
    bf1 = f1b
    bf2 = f2b - f2b.mean(1, keepdims=True)

    cW = f32(inputs["cross_in_W"])
    WcvT = b16(cW[2 * D:].T)
    bcv = f32(inputs["cross_in_b"])[2 * D:]
    WcoT = b16(f32(inputs["cross_out_W"]).T)
    bco = f32(inputs["cross_out_b"])

    # ---- trunk bias pack: [nb, 128] fp32 -> sbuf [128, nb] ----
    cols = []

    def pack(vec):
        v = f32(vec).reshape(-1, 128)
        s = len(cols)
        cols.extend(v)
        return s

    bias_idx = {
        "bp": pack(bp),
        "bv": [pack(bv[i]) for i in range(L)],
        "bo": [pack(bo0p)] + [pack(bo[i]) for i in range(1, L)],
        "bf1": [pack(bf1[i]) for i in range(L)],
        "bf2": [pack(bf2[i]) for i in range(L)],
        "bcv": pack(bcv),
        "bco": pack(bco),
    }
    TB = f32(np.stack(cols))                           # [nb, 128]

    # ---- heads ----
    W1 = f32(inputs["tp_W1"])                          # [424, 128, 256]
    b1 = f32(inputs["tp_b1"])                          # [424, 128]
    W1c = W1 - W1.mean(1, keepdims=True)
    b1c = b1 - b1.mean(1, keepdims=True)
    b1T = f32(b1c.T)                                   # [128, 424]
    # W1G[gi, k, 2*ti+c, m] = W1c[8gi+ti].T[128c+k, m]
    W1G = b16(np.transpose(
        W1c.transpose(0, 2, 1).reshape(NB8, 8, 2, 128, 128),
        (0, 3, 1, 2, 4)).reshape(NB8, 128, 16, 128))

    W2 = f32(inputs["tp_W2"])                          # [424, 64, 128]
    b2 = f32(inputs["tp_b2"])                          # [424, 64]
    W2P = W2.transpose(0, 2, 1).reshape(PAIRS, 2, 128, 64)  # [212,2,128,64]
    W2Pp = np.zeros((NPB * 8, 2, 128, 64), np.float32)
    W2Pp[:PAIRS] = W2P
    # W2G[gi, k, qi, e, m]
    W2G = b16(np.transpose(
        W2Pp.reshape(NPB, 8, 2, 128, 64), (0, 3, 1, 2, 4)
    ).reshape(NPB, 128, 1024))

    b2G = np.zeros((NG, 32, 16, 128), np.float32)
    W3 = f32(inputs["tp_W3"])                          # [424, 64]
    b3 = f32(inputs["tp_b3"])                          # [424]
    W3G = np.zeros((NG, 128, 16, 32), np.float32)
    b3B = np.zeros((32, NG), np.float32)
    for t in range(T):
        g, lt = t // 32, t % 32
        q, e = lt // 2, lt % 2
        b2G[g, lt, q, 64 * e:64 * e + 64] = b2[t]
        W3G[g, 64 * e:64 * e + 64, q, lt] = W3[t]
        b3B[lt, g] = b3[t]
    b2G = b16(b2G.reshape(NG, 32, 2048))
    W3G = b16(W3G.reshape(NG, 128, 512))
    b3B = f32(b3B)

    sel32 = b16(np.broadcast_to(np.eye(32, dtype=np.float32),
                                (128, 32, 32)).reshape(128, 1024))

    shared = {
        "WpT": WpT, "wpmC": wpmC,
        "WvT": b16(WvT), "WoT": b16(WoT),
        "Wf1T": b16(Wf1T), "Wf2T": b16(Wf2T),
        "WcvT": WcvT, "WcoT": WcoT, "TB": TB,
        "W1G": W1G, "b1T": b1T, "W2G": W2G, "b2G": b2G,
        "W3G": W3G, "b3B": b3B, "sel32": sel32,
    }
    in_maps = []
    for c in range(NCORES):
        m = dict(shared)
        m["xT"] = b16(x[c * BL:(c + 1) * BL].T)        # [512, 512]
        in_maps.append(m)
    return in_maps, TB.shape[0], bias_idx


def _build(nb, bias_idx):
    import concourse.bass as bass
    import concourse.mybir as mybir
    import concourse.tile as tile
    from concourse import bacc
    from concourse.masks import make_identity

    f32 = mybir.dt.float32
    bf = mybir.dt.bfloat16
    Alu = mybir.AluOpType
    Act = mybir.ActivationFunctionType
    ts = bass.ts

    nc = bacc.Bacc(None, target_bir_lowering=False)
    dr = lambda name, shape, dt=bf: nc.dram_tensor(name, shape, dt,
                                                   kind="ExternalInput")
    xT = dr("xT", [DIN, BL])
    WpT = dr("WpT", [DIN, D])
    wpmC = dr("wpmC", [128, 4])
    WvT = dr("WvT", [L, D, D])
    WoT = dr("WoT", [L, D, D])
    Wf1T = dr("Wf1T", [L, D, 4 * D])
    Wf2T = dr("Wf2T", [L, 4 * D, D])
    WcvT = dr("WcvT", [D, D])
    WcoT = dr("WcoT", [D, D])
    TB = dr("TB", [nb, 128], f32)
    W1G = dr("W1G", [NB8, 128, 16, 128])
    b1T = dr("b1T", [128, T], f32)
    W2G = dr("W2G", [NPB, 128, 1024])
    b2G = dr("b2G", [NG, 32, 2048])
    W3G = dr("W3G", [NG, 128, 512])
    b3B = dr("b3B", [32, NG], f32)
    sel32 = dr("sel32", [128, 1024])
    out = nc.dram_tensor("out", [BL, T], f32, kind="ExternalOutput")

    from contextlib import ExitStack

    with tile.TileContext(nc) as tc, ExitStack() as stack:
        consts = stack.enter_context(tc.tile_pool(name="consts", bufs=1))

        tb_sb = consts.tile([128, nb], f32, tag="tb")
        nc.gpsimd.dma_start(out=tb_sb, in_=TB.rearrange("n p -> p n"))
        b1_sb = consts.tile([128, T], f32, tag="b1")
        nc.gpsimd.dma_start(out=b1_sb, in_=b1T[:, :])
        b3_sb = consts.tile([32, NG], f32, tag="b3")
        nc.gpsimd.dma_start(out=b3_sb, in_=b3B[:, :])
        sel_sb = consts.tile([128, 32, 32], bf, tag="sel")
        nc.gpsimd.dma_start(out=sel_sb, in_=sel32.rearrange("p (j m) -> p j m", j=32))
        eps_col = consts.tile([128, 1], f32, tag="eps")
        nc.vector.memset(eps_col, EPS)
        ones_b = consts.tile([128, 1], bf, tag="ones")
        nc.vector.memset(ones_b, 1.0)
        idnb = consts.tile([128, 128], bf, tag="idnb")
        make_identity(nc, idnb)
        idnf = consts.tile([128, 128], f32, tag="idnf")
        make_identity(nc, idnf)
        out_sb = [consts.tile([128, T], f32, tag=f"ob{i}", name=f"ob{i}")
                  for i in range(4)]
        hcs = [consts.tile([128, BL], bf, tag=f"hc{m}", name=f"hc{m}")
               for m in range(2)]

        def bias_col(idx, m=0):
            return tb_sb[:, idx + m:idx + m + 1]

        # ================= trunk =================
        with tc.tile_pool(name="twt", bufs=2) as twt, \
             tc.tile_pool(name="tact", bufs=3) as tact, \
             tc.tile_pool(name="hpool", bufs=3) as hpool, \
             tc.tile_pool(name="tps", bufs=2, space="PSUM") as tps, \
             tc.tile_pool(name="tpss", bufs=2, space="PSUM") as tpss, \
             tc.tile_pool(name="tcps", bufs=1, space="PSUM") as tcps:

            xs = twt.tile([128, 4, BL], bf, tag="x")
            nc.gpsimd.dma_start(out=xs, in_=xT.rearrange("(c k) b -> k c b", c=4))
            wp = twt.tile([128, 4, D], bf, tag="wp")
            nc.gpsimd.dma_start(out=wp, in_=WpT.rearrange("(c k) m -> k c m", c=4))
            wpm_sb = twt.tile([128, 4], bf, tag="wpm")
            nc.gpsimd.dma_start(out=wpm_sb, in_=wpmC[:, :])

            # c = wpm . x  (layer-0 LN mean correction)
            cps = tcps.tile([1, BL], f32, tag="cps")
            for k in range(4):
                nc.tensor.matmul(cps, wpm_sb[:, k:k + 1], xs[:, k],
                                 start=(k == 0), stop=(k == 3))
            c_sb = tact.tile([1, BL], f32, tag="c1")
            nc.scalar.activation(c_sb, cps, Act.Identity, bias=0.0, scale=1.0)
            c_bc = consts.tile([128, BL], f32, tag="cbc")
            nc.gpsimd.partition_broadcast(c_bc, c_sb, channels=128)

            # h0 = Wp.T @ x + bp   (raw, uncentered)
            hps = tps.tile([128, 2, BL], f32, tag="mm")
            for m in range(2):
                for k in range(4):
                    nc.tensor.matmul(hps[:, m, :], wp[:, k, ts(m, 128)], xs[:, k],
                                     start=(k == 0), stop=(k == 3))
            h = []
            for m in range(2):
                ht = hpool.tile([128, BL], bf, tag=f"h{m}", name=f"h{m}")
                nc.scalar.activation(ht, hps[:, m, :], Act.Identity,
                                     bias=bias_col(bias_idx["bp"], m), scale=1.0)
                h.append(ht)

            def layer_norm(yps, bcol_idx, layer0=False):
                """yps: [128, 2, BL] psum holding residual+proj (no bias).
                Returns normalized bf16 tiles h[2]."""
                yp = []
                for m in range(2):
                    yt = tact.tile([128, BL], bf, tag=f"yp{m}", name="yp")
                    if layer0:
                        nc.vector.scalar_tensor_tensor(
                            out=yt, in0=yps[:, m, :],
                            scalar=bias_col(bcol_idx, m), in1=c_bc,
                            op0=Alu.add, op1=Alu.subtract)
                    else:
                        nc.scalar.activation(yt, yps[:, m, :], Act.Identity,
                                             bias=bias_col(bcol_idx, m), scale=1.0)
                    yp.append(yt)
                ssq = tpss.tile([1, BL], f32, tag="ssq")
                for m in range(2):
                    sq = tact.tile([128, BL], bf, tag="sq")
                    nc.vector.tensor_tensor(out=sq, in0=yp[m], in1=yp[m],
                                            op=Alu.mult)
                    nc.tensor.matmul(ssq, ones_b, sq,
                                     start=(m == 0), stop=(m == 1))
                sd = tact.tile([1, BL], f32, tag="sd")
                nc.scalar.activation(sd, ssq, Act.Sqrt,
                                     bias=eps_col[0:1], scale=1.0 / D)
                r = tact.tile([1, BL], f32, tag="rr")
                nc.vector.reciprocal_approx_fast(out=r, in_=sd)
                rb = tact.tile([128, BL], f32, tag="rb")
                nc.gpsimd.partition_broadcast(rb, r, channels=128)
                hn = []
                for m in range(2):
                    ht = hpool.tile([128, BL], bf, tag=f"h{m}", name=f"hn{m}")
                    nc.vector.tensor_tensor(out=ht, in0=yp[m], in1=rb, op=Alu.mult)
                    hn.append(ht)
                return hn

            for i in range(L):
                # attention == out_proj(v_proj(h)); residual via identity mm
                wv = twt.tile([128, 2, D], bf, tag="wv")
                nc.gpsimd.dma_start(out=wv, in_=WvT[i].rearrange(
                    "(c k) m -> k c m", c=2))
                vps = tps.tile([128, 2, BL], f32, tag="mm")
                for m in range(2):
                    for k in range(2):
                        nc.tensor.matmul(vps[:, m, :], wv[:, k, ts(m, 128)], h[k],
                                         start=(k == 0), stop=(k == 1))
                v = []
                for m in range(2):
                    vt = tact.tile([128, BL], bf, tag=f"v{m}")
                    nc.scalar.activation(vt, vps[:, m, :], Act.Identity,
                                         bias=bias_col(bias_idx["bv"][i], m),
                                         scale=1.0)
                    v.append(vt)
                wo = twt.tile([128, 2, D], bf, tag="wo")
                nc.gpsimd.dma_start(out=wo, in_=WoT[i].rearrange(
                    "(c k) m -> k c m", c=2))
                yps = tps.tile([128, 2, BL], f32, tag="mm")
                for m in range(2):
                    for k in range(2):
                        nc.tensor.matmul(yps[:, m, :], wo[:, k, ts(m, 128)], v[k],
                                         start=(k == 0), stop=False)
                    nc.tensor.matmul(yps[:, m, :], idnb, h[m],
                                     start=False, stop=True)
                h = layer_norm(yps, bias_idx["bo"][i], layer0=(i == 0))

                # feed-forward
                w1 = twt.tile([128, 2, 4 * D], bf, tag="wf1")
                nc.gpsimd.dma_start(out=w1, in_=Wf1T[i].rearrange(
                    "(c k) m -> k c m", c=2))
                g = []
                for gm in range(4):
                    gps = tps.tile([128, 2, BL], f32, tag="mm")
                    for half in range(2):
                        m = 2 * gm + half
                        for k in range(2):
                            nc.tensor.matmul(gps[:, half, :],
                                             w1[:, k, ts(m, 128)], h[k],
                                             start=(k == 0), stop=(k == 1))
                        gt = tact.tile([128, BL], bf, tag=f"g{m}", name=f"g{m}")
                        nc.scalar.activation(gt, gps[:, half, :], Act.Gelu,
                                             bias=bias_col(bias_idx["bf1"][i], m),
                                             scale=1.0)
                        g.append(gt)
                w2 = twt.tile([128, 8, D], bf, tag="wf2")
                nc.gpsimd.dma_start(out=w2, in_=Wf2T[i].rearrange(
                    "(c k) m -> k c m", c=8))
                yps = tps.tile([128, 2, BL], f32, tag="mm")
                for m in range(2):
                    for k in range(8):
                        nc.tensor.matmul(yps[:, m, :], w2[:, k, ts(m, 128)], g[k],
                                         start=(k == 0), stop=False)
                    nc.tensor.matmul(yps[:, m, :], idnb, h[m],
                                     start=False, stop=True)
                h = layer_norm(yps, bias_idx["bf2"][i])

            # cross attention (residual, no LN)
            wv = twt.tile([128, 2, D], bf, tag="wv")
            nc.gpsimd.dma_start(out=wv, in_=WcvT.rearrange("(c k) m -> k c m", c=2))
            vps = tps.tile([128, 2, BL], f32, tag="mm")
            for m in range(2):
                for k in range(2):
                    nc.tensor.matmul(vps[:, m, :], wv[:, k, ts(m, 128)], h[k],
                                     start=(k == 0), stop=(k == 1))
            v = []
            for m in range(2):
                vt = tact.tile([128, BL], bf, tag=f"v{m}")
                nc.scalar.activation(vt, vps[:, m, :], Act.Identity,
                                     bias=bias_col(bias_idx["bcv"], m), scale=1.0)
                v.append(vt)
            wo = twt.tile([128, 2, D], bf, tag="wo")
            nc.gpsimd.dma_start(out=wo, in_=WcoT.rearrange("(c k) m -> k c m", c=2))
            yps = tps.tile([128, 2, BL], f32, tag="mm")
            for m in range(2):
                for k in range(2):
                    nc.tensor.matmul(yps[:, m, :], wo[:, k, ts(m, 128)], v[k],
                                     start=(k == 0), stop=False)
                nc.tensor.matmul(yps[:, m, :], idnb, h[m], start=False, stop=True)
                nc.scalar.activation(hcs[m], yps[:, m, :], Act.Identity,
                                     bias=bias_col(bias_idx["bco"], m), scale=1.0)

        # ================= heads =================
        with tc.tile_pool(name="w1p", bufs=3) as w1p, \
             tc.tile_pool(name="w2p", bufs=2) as w2p, \
             tc.tile_pool(name="w3p", bufs=2) as w3p, \
             tc.tile_pool(name="b2p", bufs=2) as b2p, \
             tc.tile_pool(name="sqp", bufs=4) as sqp, \
             tc.tile_pool(name="Rp", bufs=40) as Rp, \
             tc.tile_pool(name="R2p", bufs=3) as R2p, \
             tc.tile_pool(name="grp", bufs=2) as grp, \
             tc.tile_pool(name="Tps", bufs=3, space="PSUM") as Tps, \
             tc.tile_pool(name="Sps", bufs=1, space="PSUM") as Sps, \
             tc.tile_pool(name="Zps", bufs=2, space="PSUM") as Zps, \
             tc.tile_pool(name="Ops", bufs=1, space="PSUM") as Ops, \
             tc.tile_pool(name="Pps", bufs=1, space="PSUM") as Pps:

            w1t = None
            for g in range(NG):
                gs = min(32, T - 32 * g)
                npair = gs // 2
                w3t = w3p.tile([128, 16, 32], bf, tag="w3", name="w3")
                nc.gpsimd.dma_start(out=w3t, in_=W3G[g].rearrange(
                    "p (q m) -> p q m", q=16))
                b2t = b2p.tile([32, 16, 128], bf, tag="b2", name="b2")
                nc.gpsimd.dma_start(out=b2t, in_=b2G[g].rearrange(
                    "p (q m) -> p q m", q=16))

                # phase A: T, sq, R, stats for all targets in group
                ssq = Sps.tile([32, BL], f32, tag="ssq", name="ssq")
                Rlist = []
                for ti in range(gs):
                    t = 32 * g + ti
                    if t % 8 == 0:
                        w1t = w1p.tile([128, 16, 128], bf, tag="w1", name="w1")
                        nc.gpsimd.dma_start(out=w1t, in_=W1G[t // 8])
                    tps_ = Tps.tile([128, BL], f32, tag="T", name="Tps")
                    for k in range(2):
                        nc.tensor.matmul(tps_, w1t[:, 2 * (t % 8) + k, :], hcs[k],
                                         start=(k == 0), stop=(k == 1))
                    sq = sqp.tile([128, BL], bf, tag="sq", name="sq")
                    nc.scalar.activation(sq, tps_, Act.Square,
                                         bias=b1_sb[:, t:t + 1], scale=1.0)
                    R = Rp.tile([128, BL], bf, tag="R", name="R")
                    nc.vector.tensor_scalar(
                        out=R, in0=tps_, scalar1=b1_sb[:, t:t + 1],
                        scalar2=0.0, op0=Alu.add, op1=Alu.max)
                    nc.tensor.matmul(ssq, sel_sb[:, ti, :], sq,
                                     start=(ti == 0), stop=(ti == gs - 1))
                    Rlist.append(R)

                # group stats: sd = sqrt(var+eps), rstd = 1/sd
                sdf = grp.tile([32, BL], f32, tag="sdf", name="sdf")
                nc.scalar.activation(sdf, ssq, Act.Sqrt,
                                     bias=eps_col[0:32], scale=1.0 / 128)
                sdb = grp.tile([32, BL], bf, tag="sdb", name="sdb")
                nc.vector.tensor_copy(out=sdb, in_=sdf)
                rstd = grp.tile([32, BL], f32, tag="rst", name="rstd")
                nc.vector.reciprocal_approx_fast(out=rstd, in_=sdf)

                # phase B: Z (+ b2 (x) sd), relu, W3 accumulation
                o3g = Ops.tile([32, BL], f32, tag="o3g", name="o3g")
                for q in range(npair):
                    p = 16 * g + q
                    if p % 8 == 0:
                        w2t = w2p.tile([128, 8, 2, 64], bf, tag="w2", name="w2")
                        nc.gpsimd.dma_start(out=w2t, in_=W2G[p // 8].rearrange(
                            "p (q e m) -> p q e m", q=8, e=2))
                    zps = Zps.tile([128, BL], f32, tag="z", name="zps")
                    qi = p % 8
                    # rank-1 b2 (x) sd first: full-tile start=True makes the
                    # has_written state unambiguous for the partial writes.
                    nc.tensor.matmul(zps, b2t[:, q, :], sdb,
                                     start=True, stop=False)
                    nc.tensor.matmul(zps[0:64], w2t[:, qi, 0, :], Rlist[2 * q],
                                     start=False, stop=False)
                    nc.tensor.matmul(zps[64:128], w2t[:, qi, 1, :],
                                     Rlist[2 * q + 1], start=False, stop=True)
                    R2 = R2p.tile([128, BL], bf, tag="R2", name="R2")
                    if q % 2 == 0:
                        nc.scalar.activation(R2, zps, Act.Relu, bias=0.0,
                                             scale=1.0)
                    else:
                        nc.vector.tensor_scalar(out=R2, in0=zps, scalar1=0.0,
                                                scalar2=None, op0=Alu.max)
                    nc.tensor.matmul(o3g, w3t[:, q, :], R2,
                                     start=(q == 0), stop=(q == npair - 1))

                # final: scale by rstd, add b3, transpose to [batch, target]
                o3u = grp.tile([32, BL], bf, tag="o3u", name="o3u")
                nc.vector.tensor_tensor(out=o3u, in0=o3g, in1=rstd, op=Alu.mult)
                o3f = grp.tile([32, BL], f32, tag="o3f", name="o3f")
                nc.scalar.activation(o3f, o3u, Act.Identity,
                                     bias=b3_sb[0:32, g:g + 1], scale=1.0)
                for bc in range(4):
                    tp = Pps.tile([128, 32], f32, tag="tp", name="tp")
                    nc.tensor.transpose(tp[:, 0:gs],
                                        o3f[0:gs, ts(bc, 128)], idnf[0:gs, 0:gs])
                    nc.vector.tensor_copy(
                        out=out_sb[bc][:, 32 * g:32 * g + gs],
                        in_=tp[:, 0:gs])
            for bc in range(4):
                nc.gpsimd.dma_start(out=out[ts(bc, 128)], in_=out_sb[bc])

    nc.compile()
    return nc


def kernel(**inputs):
    from concourse.bass_utils import run_bass_kernel_spmd

    in_maps, nb, bias_idx = _prep(inputs)
    if "nc" not in _cache:
        _cache["nc"] = _build(nb, bias_idx)
    nc = _cache["nc"]
    import os
    res = run_bass_kernel_spmd(
        nc, in_maps, core_ids=list(range(NCORES)),
        trace=bool(int(os.environ.get("KTRACE", "0"))))
    _cache["last_result"] = res
    outs = [np.asarray(r["out"], dtype=np.float32) for r in res.results]
    return np.concatenate(outs, axis=0)
